# revision 12
# baseline (speedup 1.0000x reference)
"""Trainium2 Bass kernel for spatial self-attention (nn_Attention_90615220011343).

Module math (per batch b):
    qkv = x @ w_qkv            x:[N=4096, C=256], w_qkv:[256, 384]
    q,k,v -> heads (4 heads, dim 32)
    sim = (q*ds^-0.5) @ k^T    per head: [4096, 4096]
    attn = softmax(sim, -1)
    out = attn @ v             -> [N, 128]
    y = out @ w_out + b_out    -> [N, 256]

Sharding: 8 cores = 4 batches x 2 head-pairs. Core c -> batch c//2,
heads {2*(c%2), 2*(c%2)+1}. Each core computes a partial y (its two
heads' contribution); host sums the pair and adds b_out.

Per-core kernel layout strategy (all on-chip, no collectives):
  - x^T [2x128, 4096] via PE transposes (contraction dim C on partitions).
  - q^T replicated 4x along partitions (via host-tiled wq columns) so the
    K=32 sim matmuls can be packed 4-per-PE-pass with row tiling.
  - k^T stored "interleaved-stacked": j-chunk c (128 tokens) lives at
    partition base 32*(c%4), column block c//4. Any 3-4 consecutive
    chunks occupy distinct row-groups -> one row-tiled matmul group.
  - sim^T computed in [j, i] layout (j on partitions) so softmax exp is a
    pure elementwise pass (values are N(0,1); max-subtraction skipped -
    exp never overflows) and attn@v needs no transpose.
  - attn@v: lhsT = [v | 1] (ones column rides along, M=33) so row 32 of
    the psum accumulator is the softmax denominator for free.
  - normalization folded to the very end: y_h = (out_h @ w_out_h) scaled
    per-partition by 1/den_h, summed over the core's 2 heads on DVE.
"""

import numpy as np

HEADS = 4
DH = 32
N = 4096
C = 256
P = 128
NCH = 32  # number of 128-token j-chunks
ITILES = 8  # i tiles of 512
GROUPS = [4, 3, 4, 3, 4, 3, 4, 3, 4]  # j-chunks per sim/exp group (A/B slabs)

_CACHED = {}


def _build_nc():
    import concourse.bass as bass
    import concourse.mybir as mybir
    from concourse.tile import TileContext
    from concourse.masks import make_identity

    FP = mybir.dt.float32
    FR = mybir.dt.float32r
    AF = mybir.ActivationFunctionType
    ALU = mybir.AluOpType

    nc = bass.Bass(target_bir_lowering=False)
    x_d = nc.declare_dram_parameter("x", [N, C], FP, isOutput=False)
    wq_d = nc.declare_dram_parameter("wq", [C, 2 * P], FP, isOutput=False)
    wk_d = nc.declare_dram_parameter("wk", [C, 64], FP, isOutput=False)
    wv_d = nc.declare_dram_parameter("wv", [C, 64], FP, isOutput=False)
    wo_d = nc.declare_dram_parameter("wo", [64, C], FP, isOutput=False)
    y_d = nc.declare_dram_parameter("y", [N, C], FP, isOutput=True)

    with TileContext(nc) as tc:
        with (
            tc.tile_pool(name="const", bufs=1) as constp,
            tc.tile_pool(name="xin", bufs=10) as xinp,
            tc.tile_pool(name="big", bufs=1) as bigp,
            tc.tile_pool(name="exp", bufs=3) as expp,
            tc.tile_pool(name="ytmp", bufs=4) as ytmpp,
            tc.tile_pool(name="psA", bufs=1, space="PSUM") as psA,
            tc.tile_pool(name="psB", bufs=1, space="PSUM") as psB,
            tc.tile_pool(name="psV", bufs=1, space="PSUM") as psV,
        ):
            ident = constp.tile([P, P], FP, tag="ident")
            make_identity(nc, ident[:])

            # ---- persistent SBUF tensors ----
            xT = [bigp.tile([P, N], FR, tag=f"xT{cc}", name=f"xT{cc}") for cc in range(2)]
            qrep = [bigp.tile([P, N], FR, tag=f"qrep{h}", name=f"qrep{h}") for h in range(2)]
            karr = [bigp.tile([P, N // 4], FR, tag=f"karr{h}", name=f"karr{h}") for h in range(2)]
            vaug = [bigp.tile([P, 33 * NCH], FR, tag=f"vaug{h}", name=f"vaug{h}") for h in range(2)]
            outT = bigp.tile([64, N], FR, tag="outT")
            # softmax denominators: head h at partition 32*h
            denrow = bigp.tile([33, N], FP, tag="denrow")
            rden = bigp.tile([P, 64], FP, tag="rden")
            wq_sb = bigp.tile([P, 2, 2 * P], FR, tag="wq")
            wk_sb = bigp.tile([P, 2, 64], FR, tag="wk")
            wv_sb = bigp.tile([P, 2, 64], FR, tag="wv")
            wo_sb = bigp.tile([64, C], FR, tag="wo")

            # ---- weight + x loads (stage fp32, round to fp32r on DVE) ----
            wq_st = bigp.tile([P, 2, 2 * P], FP, tag="wq_st")
            wk_st = bigp.tile([P, 2, 64], FP, tag="wk_st")
            wv_st = bigp.tile([P, 2, 64], FP, tag="wv_st")
            wo_st = bigp.tile([64, C], FP, tag="wo_st")
            for cc in range(2):
                nc.sync.dma_start(out=wq_st[:, cc, :], in_=wq_d[cc * P:(cc + 1) * P, :])
                nc.sync.dma_start(out=wk_st[:, cc, :], in_=wk_d[cc * P:(cc + 1) * P, :])
                nc.sync.dma_start(out=wv_st[:, cc, :], in_=wv_d[cc * P:(cc + 1) * P, :])
            nc.sync.dma_start(out=wo_st[:], in_=wo_d[:])
            nc.vector.tensor_copy(out=wq_sb[:], in_=wq_st[:])
            nc.vector.tensor_copy(out=wk_sb[:], in_=wk_st[:])
            nc.vector.tensor_copy(out=wv_sb[:], in_=wv_st[:])
            nc.vector.tensor_copy(out=wo_sb[:], in_=wo_st[:])

            # ---- x load + transpose to xT ----
            # rounds: (pool, tag, nk list); each slab holds both c-chunks of
            # its nk's interleaved: [nk0/cc0, nk0/cc1, nk1/cc0, ...]
            tp_rounds = [
                (psA, "A", list(range(0, 8))),
                (psB, "B", list(range(8, 14))),
                (psA, "A", list(range(14, 22))),
                (psB, "B", list(range(22, 28))),
                (psA, "A", list(range(28, 32))),
            ]
            for pool, tag, nks in tp_rounds:
                L = 2048 if tag == "A" else 1536
                slab = pool.tile([P, L], FP, tag=tag)
                for i, nk in enumerate(nks):
                    xt = xinp.tile([P, C], FP, tag="xt")
                    nc.sync.dma_start(out=xt[:], in_=x_d[P * nk:P * (nk + 1), :])
                    for cc in range(2):
                        nc.tensor.transpose(
                            slab[:, 256 * i + P * cc: 256 * i + P * (cc + 1)],
                            xt[:, P * cc:P * (cc + 1)],
                            ident[:],
                        )
                n = len(nks)
                sv = slab[:].rearrange("p (k c f) -> p k c f", c=2, f=P)
                for cc in range(2):
                    nc.vector.tensor_copy(
                        out=xT[cc][:, P * nks[0]: P * (nks[0] + n)],
                        in_=sv[:, 0:n, cc, :],
                    )

            # ---- qkv projections ----
            def qrep_rounds(h):
                for pool, tag, it0, nits in (
                    (psA, "A", 0, 4), (psB, "B", 4, 3), (psA, "A", 7, 1),
                ):
                    L = 2048 if tag == "A" else 1536
                    slab = pool.tile([P, L], FP, tag=tag)
                    for cc in range(2):
                        for r in range(nits):
                            it = it0 + r
                            nc.tensor.matmul(
                                slab[:, 512 * r: 512 * (r + 1)],
                                lhsT=wq_sb[:, cc, P * h: P * (h + 1)],
                                rhs=xT[cc][:, 512 * it: 512 * (it + 1)],
                                start=(cc == 0), stop=(cc == 1),
                            )
                    nc.vector.tensor_copy(
                        out=qrep[h][:, 512 * it0: 512 * (it0 + nits)],
                        in_=slab[:, : 512 * nits],
                    )

            def karr_build(h):
                # karr[32*(c%4) : +32, 128*(c//4) : +128] = k^T of j-chunk c
                # (col-tiling is incompatible with fp32r: all matmuls write
                # partition base 0; DVE relocates to the stacked layout)
                for p_ in range(2):
                    slab = psA.tile([P, 2048], FP, tag="A")
                    for ct in range(4):
                        # rhs: j-chunks c = 4m+ct for m in [4p, 4p+4) -> strided view
                        for cc in range(2):
                            xv = xT[cc][:].rearrange(
                                "q (m t f) -> q m t f", t=4, f=P
                            )[:, 4 * p_: 4 * p_ + 4, ct, :]
                            nc.tensor.matmul(
                                slab[0:32, 512 * ct: 512 * (ct + 1)],
                                lhsT=wk_sb[:, cc, 32 * h: 32 * (h + 1)],
                                rhs=xv,
                                start=(cc == 0), stop=(cc == 1),
                            )
                    for ct in range(4):
                        nc.vector.tensor_copy(
                            out=karr[h][32 * ct: 32 * (ct + 1), 512 * p_: 512 * (p_ + 1)],
                            in_=slab[0:32, 512 * ct: 512 * (ct + 1)],
                        )

            def v_build():
                # both heads at once: psum [128, 64*(k%..)] chunks
                slab = psA.tile([P, 2048], FP, tag="A")
                for k in range(NCH):
                    for cc in range(2):
                        nc.tensor.matmul(
                            slab[:, 64 * k: 64 * (k + 1)],
                            lhsT=xT[cc][:, P * k: P * (k + 1)],
                            rhs=wv_sb[:, cc, :],
                            start=(cc == 0), stop=(cc == 1),
                        )
                sv = slab[:].rearrange("p (k d) -> p k d", d=64)
                ones_st = bigp.tile([P, NCH], FP, tag="ones_st")
                nc.gpsimd.memset(ones_st[:], 1.0)
                for h in range(2):
                    vv = vaug[h][:].rearrange("p (k e) -> p k e", e=33)
                    nc.vector.tensor_copy(out=vv[:, :, 32], in_=ones_st[:])
                    nc.vector.tensor_copy(
                        out=vv[:, :, 0:32], in_=sv[:, :, 32 * h: 32 * (h + 1)]
                    )

            qrep_rounds(0)
            karr_build(0)
            v_build()
            qrep_rounds(1)
            karr_build(1)

            # ---- attention ----
            for h in range(2):
                vv = vaug[h][:].rearrange("p (k e) -> p k e", e=33)
                for it in range(ITILES):
                    i0 = 512 * it
                    av = psV.tile([P, 512], FP, tag="V")
                    cstart = 0
                    for gsz in GROUPS:
                        pool, tag, L = (psA, "A", 2048) if gsz == 4 else (psB, "B", 1536)
                        slab = pool.tile([P, L], FP, tag=tag)
                        for r in range(gsz):
                            c = cstart + r
                            rt = c % 4
                            nc.tensor.matmul(
                                slab[:, 512 * r: 512 * (r + 1)],
                                lhsT=karr[h][32 * rt: 32 * (rt + 1), P * (c // 4): P * (c // 4 + 1)],
                                rhs=qrep[h][32 * rt: 32 * (rt + 1), i0: i0 + 512],
                                start=True, stop=True,
                                tile_position=(32 * rt, 0),
                            )
                        eslab = expp.tile([P, L], FR, tag="E")
                        nc.scalar.activation(eslab[:], slab[:], AF.Exp)
                        for r in range(gsz):
                            c = cstart + r
                            nc.tensor.matmul(
                                av[0:33, :],
                                lhsT=vv[:, c, :],
                                rhs=eslab[:, 512 * r: 512 * (r + 1)],
                                start=(c == 0), stop=(c == NCH - 1),
                                skip_group_check=True,
                            )
                        cstart += gsz
                    nc.vector.tensor_copy(out=outT[32 * h: 32 * h + 32, i0: i0 + 512], in_=av[0:32, :])
                    nc.vector.tensor_copy(out=denrow[32 * h: 32 * h + 1, i0: i0 + 512], in_=av[32:33, :])

                # denominator row -> column layout, reciprocal
                dslab = psV.tile([P, 512], FP, tag="V")
                for t in range(NCH):
                    nc.tensor.transpose(
                        dslab[:, t: t + 1],
                        denrow[32 * h: 32 * h + 1, P * t: P * (t + 1)],
                        ident[32 * h: 32 * h + 1, 32 * h: 32 * h + 1],
                    )
                nc.vector.reciprocal(out=rden[:, 32 * h: 32 * h + 32], in_=dslab[:, 0:32])

            # ---- output projection + per-head normalize + combine ----
            for k in range(NCH):
                ya = psA.tile([P, 2048], FP, tag="A")
                yb = psB.tile([P, 1536], FP, tag="B")
                nc.tensor.matmul(
                    ya[:, 0:C], lhsT=outT[0:32, P * k: P * (k + 1)],
                    rhs=wo_sb[0:32, :],
                    start=True, stop=True, tile_position=(0, 0),
                )
                nc.tensor.matmul(
                    yb[:, 0:C], lhsT=outT[32:64, P * k: P * (k + 1)],
                    rhs=wo_sb[32:64, :],
                    start=True, stop=True, tile_position=(32, 0),
                )
                t0 = ytmpp.tile([P, C], FP, tag="t0")
                nc.vector.tensor_scalar_mul(t0[:], ya[:, 0:C], rden[:, k: k + 1])
                yo = ytmpp.tile([P, C], FP, tag="yo")
                nc.vector.scalar_tensor_tensor(
                    out=yo[:], in0=yb[:, 0:C], scalar=rden[:, 32 + k: 33 + k],
                    in1=t0[:], op0=ALU.mult, op1=ALU.add,
                )
                nc.sync.dma_start(out=y_d[P * k: P * (k + 1), :], in_=yo[:])

    _split_excess_waits(nc, mybir)
    return nc


def _split_excess_waits(nc, mybir, maxw=1, carrier_cap=1):
    """walrus codegen allows few semaphore waits per engine instruction.

    Tile's scheduler can emit 3-4 on one matmul. Hoist the excess onto
    InstEventSemaphore carriers inserted immediately before the instruction
    on the same engine queue (queue is FIFO, so waiting in the carrier is
    equivalent; no reordering so no deadlock risk).
    """
    skip = {
        "InstEventSemaphore", "InstCall",
        "InstUnconditionalBranch", "InstISA", "InstRegisterMove",
    }
    for f in nc.m.functions:
        for blk in f.blocks:
            idx = 0
            while idx < len(blk.instructions):
                ins = blk.instructions[idx]
                si = getattr(ins, "sync_info", None)
                if (
                    si is not None and si.on_wait and len(si.on_wait) > maxw
                    and type(ins).__name__ not in skip
                ):
                    waits = list(si.on_wait)
                    keep, excess = waits[:maxw], waits[maxw:]
                    n_ins = 0
                    for i in range(0, len(excess), carrier_cap):
                        ev = mybir.InstEventSemaphore(
                            name=nc.get_next_instruction_name(),
                            engine=ins.engine,
                            ins=[], outs=[],
                            sync_info=mybir.SyncInfo(
                                on_wait=excess[i:i + carrier_cap], on_update=[]
                            ),
                        )
                        nc.register_instruction(ev)
                        blk.instructions.insert(idx + n_ins, ev)
                        n_ins += 1
                    ins.sync_info = mybir.SyncInfo(
                        on_wait=keep, on_update=list(si.on_update or [])
                    )
                    idx += n_ins
                idx += 1
    return nc


def get_nc():
    if "nc" not in _CACHED:
        _CACHED["nc"] = _build_nc()
    return _CACHED["nc"]


def make_in_maps(x, w_qkv, w_out):
    """Host-side sharding: core c -> batch c//2, heads (c%2)*2, (c%2)*2+1."""
    B = x.shape[0]
    xf = np.ascontiguousarray(x.reshape(B, N, C))
    scale = DH ** -0.5
    in_maps = []
    for core in range(8):
        b, hp = core // 2, core % 2
        h0, h1 = 2 * hp, 2 * hp + 1
        wq = np.concatenate(
            [np.tile(w_qkv[:, h * DH:(h + 1) * DH] * scale, (1, 4)) for h in (h0, h1)],
            axis=1,
        )  # [256, 256]
        wk = np.concatenate(
            [w_qkv[:, 128 + h * DH: 128 + (h + 1) * DH] for h in (h0, h1)], axis=1
        )  # [256, 64]
        wv = np.concatenate(
            [w_qkv[:, 256 + h * DH: 256 + (h + 1) * DH] for h in (h0, h1)], axis=1
        )  # [256, 64]
        wo = np.concatenate(
            [w_out[h * DH:(h + 1) * DH, :] for h in (h0, h1)], axis=0
        )  # [64, 256]
        in_maps.append({
            "x": np.ascontiguousarray(xf[b]),
            "wq": np.ascontiguousarray(wq.astype(np.float32)),
            "wk": np.ascontiguousarray(wk.astype(np.float32)),
            "wv": np.ascontiguousarray(wv.astype(np.float32)),
            "wo": np.ascontiguousarray(wo.astype(np.float32)),
        })
    return in_maps


def kernel(x, w_qkv, w_out, b_out):
    from concourse.bass_utils import run_bass_kernel_spmd

    nc = get_nc()
    in_maps = make_in_maps(
        np.asarray(x, dtype=np.float32),
        np.asarray(w_qkv, dtype=np.float32),
        np.asarray(w_out, dtype=np.float32),
    )
    res = run_bass_kernel_spmd(nc, in_maps, list(range(8))).results
    B, H, W = 4, 64, 64
    y = np.empty((B, N, C), dtype=np.float32)
    for b in range(B):
        y[b] = res[2 * b]["y"] + res[2 * b + 1]["y"]
    y += np.asarray(b_out, dtype=np.float32)
    return y.reshape(B, H, W, C)


# revision 13
# speedup vs baseline: 1.0281x; 1.0281x over previous
"""Trainium2 Bass kernel for spatial self-attention (nn_Attention_90615220011343).

Module math (per batch b):
    qkv = x @ w_qkv            x:[N=4096, C=256], w_qkv:[256, 384]
    q,k,v -> heads (4 heads, dim 32)
    sim = (q*ds^-0.5) @ k^T    per head: [4096, 4096]
    attn = softmax(sim, -1)
    out = attn @ v             -> [N, 128]
    y = out @ w_out + b_out    -> [N, 256]

Sharding: 8 cores = 4 batches x 2 head-pairs. Core c -> batch c//2,
heads {2*(c%2), 2*(c%2)+1}. Each core computes a partial y (its two
heads' contribution); host sums the pair and adds b_out.

Per-core kernel layout strategy (all on-chip, no collectives):
  - x^T [2x128, 4096] via PE transposes (contraction dim C on partitions).
  - q^T replicated 4x along partitions (via host-tiled wq columns) so the
    K=32 sim matmuls can be packed 4-per-PE-pass with row tiling.
  - k^T stored "interleaved-stacked": j-chunk c (128 tokens) lives at
    partition base 32*(c%4), column block c//4. Any 3-4 consecutive
    chunks occupy distinct row-groups -> one row-tiled matmul group.
  - sim^T computed in [j, i] layout (j on partitions) so softmax exp is a
    pure elementwise pass (values are N(0,1); max-subtraction skipped -
    exp never overflows) and attn@v needs no transpose.
  - attn@v: lhsT = [v | 1] (ones column rides along, M=33) so row 32 of
    the psum accumulator is the softmax denominator for free.
  - normalization folded to the very end: y_h = (out_h @ w_out_h) scaled
    per-partition by 1/den_h, summed over the core's 2 heads on DVE.
"""

import numpy as np

HEADS = 4
DH = 32
N = 4096
C = 256
P = 128
NCH = 32  # number of 128-token j-chunks
ITILES = 8  # i tiles of 512
GROUPS = [4, 3, 4, 3, 4, 3, 4, 3, 4]  # j-chunks per sim/exp group (A/B slabs)

_CACHED = {}


def _build_nc():
    import concourse.bass as bass
    import concourse.mybir as mybir
    from concourse.tile import TileContext
    from concourse.masks import make_identity

    FP = mybir.dt.float32
    FR = mybir.dt.float32r
    AF = mybir.ActivationFunctionType
    ALU = mybir.AluOpType

    nc = bass.Bass(target_bir_lowering=False)
    x_d = nc.declare_dram_parameter("x", [N, C], FP, isOutput=False)
    wq_d = nc.declare_dram_parameter("wq", [C, 2 * P], FP, isOutput=False)
    wk_d = nc.declare_dram_parameter("wk", [C, 64], FP, isOutput=False)
    wv_d = nc.declare_dram_parameter("wv", [C, 64], FP, isOutput=False)
    wo_d = nc.declare_dram_parameter("wo", [64, C], FP, isOutput=False)
    y_d = nc.declare_dram_parameter("y", [N, C], FP, isOutput=True)

    with TileContext(nc) as tc:
        with (
            tc.tile_pool(name="const", bufs=1) as constp,
            tc.tile_pool(name="xin", bufs=10) as xinp,
            tc.tile_pool(name="big", bufs=1) as bigp,
            tc.tile_pool(name="exp", bufs=2) as expp,
            tc.tile_pool(name="ytmp", bufs=4) as ytmpp,
            tc.tile_pool(name="psA", bufs=1, space="PSUM") as psA,
            tc.tile_pool(name="psB", bufs=1, space="PSUM") as psB,
            tc.tile_pool(name="psV", bufs=1, space="PSUM") as psV,
        ):
            ident = constp.tile([P, P], FP, tag="ident")
            make_identity(nc, ident[:])

            # ---- persistent SBUF tensors ----
            xT = [bigp.tile([P, N], FR, tag=f"xT{cc}", name=f"xT{cc}") for cc in range(2)]
            qrep = [bigp.tile([P, N], FR, tag=f"qrep{h}", name=f"qrep{h}") for h in range(2)]
            karr = [bigp.tile([P, N // 4], FR, tag=f"karr{h}", name=f"karr{h}") for h in range(2)]
            vaug = [bigp.tile([P, 33 * NCH], FR, tag=f"vaug{h}", name=f"vaug{h}") for h in range(2)]
            outT = bigp.tile([64, N], FR, tag="outT")
            # softmax denominators: head h at partition 32*h
            denrow = bigp.tile([33, N], FP, tag="denrow")
            rden = bigp.tile([P, 64], FP, tag="rden")
            wq_sb = bigp.tile([P, 2, 2 * P], FR, tag="wq")
            wk_sb = bigp.tile([P, 2, 64], FR, tag="wk")
            wv_sb = bigp.tile([P, 2, 64], FR, tag="wv")
            wo_sb = bigp.tile([64, C], FR, tag="wo")

            # ---- weight + x loads (stage fp32, round to fp32r on DVE) ----
            wq_st = bigp.tile([P, 2, 2 * P], FP, tag="wq_st")
            wk_st = bigp.tile([P, 2, 64], FP, tag="wk_st")
            wv_st = bigp.tile([P, 2, 64], FP, tag="wv_st")
            wo_st = bigp.tile([64, C], FP, tag="wo_st")
            for cc in range(2):
                nc.sync.dma_start(out=wq_st[:, cc, :], in_=wq_d[cc * P:(cc + 1) * P, :])
                nc.sync.dma_start(out=wk_st[:, cc, :], in_=wk_d[cc * P:(cc + 1) * P, :])
                nc.sync.dma_start(out=wv_st[:, cc, :], in_=wv_d[cc * P:(cc + 1) * P, :])
            nc.sync.dma_start(out=wo_st[:], in_=wo_d[:])
            nc.vector.tensor_copy(out=wq_sb[:], in_=wq_st[:])
            nc.vector.tensor_copy(out=wk_sb[:], in_=wk_st[:])
            nc.vector.tensor_copy(out=wv_sb[:], in_=wv_st[:])
            nc.vector.tensor_copy(out=wo_sb[:], in_=wo_st[:])

            # ---- x load + transpose to xT ----
            # rounds: (pool, tag, nk list); each slab holds both c-chunks of
            # its nk's interleaved: [nk0/cc0, nk0/cc1, nk1/cc0, ...]
            tp_rounds = [
                (psA, "A", list(range(0, 8))),
                (psB, "B", list(range(8, 14))),
                (psA, "A", list(range(14, 22))),
                (psB, "B", list(range(22, 28))),
                (psA, "A", list(range(28, 32))),
            ]
            for pool, tag, nks in tp_rounds:
                L = 2048 if tag == "A" else 1536
                slab = pool.tile([P, L], FP, tag=tag)
                for i, nk in enumerate(nks):
                    xt = xinp.tile([P, C], FP, tag="xt")
                    dmae = nc.sync if nk % 2 == 0 else nc.scalar
                    dmae.dma_start(out=xt[:], in_=x_d[P * nk:P * (nk + 1), :])
                    for cc in range(2):
                        nc.tensor.transpose(
                            slab[:, 256 * i + P * cc: 256 * i + P * (cc + 1)],
                            xt[:, P * cc:P * (cc + 1)],
                            ident[:],
                        )
                n = len(nks)
                sv = slab[:].rearrange("p (k c f) -> p k c f", c=2, f=P)
                for cc in range(2):
                    nc.vector.tensor_copy(
                        out=xT[cc][:, P * nks[0]: P * (nks[0] + n)],
                        in_=sv[:, 0:n, cc, :],
                    )

            # ---- qkv projections ----
            def qrep_rounds(h):
                for pool, tag, it0, nits in (
                    (psA, "A", 0, 4), (psB, "B", 4, 3), (psA, "A", 7, 1),
                ):
                    L = 2048 if tag == "A" else 1536
                    slab = pool.tile([P, L], FP, tag=tag)
                    for cc in range(2):
                        for r in range(nits):
                            it = it0 + r
                            nc.tensor.matmul(
                                slab[:, 512 * r: 512 * (r + 1)],
                                lhsT=wq_sb[:, cc, P * h: P * (h + 1)],
                                rhs=xT[cc][:, 512 * it: 512 * (it + 1)],
                                start=(cc == 0), stop=(cc == 1),
                            )
                    nc.vector.tensor_copy(
                        out=qrep[h][:, 512 * it0: 512 * (it0 + nits)],
                        in_=slab[:, : 512 * nits],
                    )

            def karr_build(h):
                # karr[32*(c%4) : +32, 128*(c//4) : +128] = k^T of j-chunk c
                # (col-tiling is incompatible with fp32r: all matmuls write
                # partition base 0; DVE relocates to the stacked layout)
                for p_ in range(2):
                    slab = psA.tile([P, 2048], FP, tag="A")
                    for ct in range(4):
                        # rhs: j-chunks c = 4m+ct for m in [4p, 4p+4) -> strided view
                        for cc in range(2):
                            xv = xT[cc][:].rearrange(
                                "q (m t f) -> q m t f", t=4, f=P
                            )[:, 4 * p_: 4 * p_ + 4, ct, :]
                            nc.tensor.matmul(
                                slab[0:32, 512 * ct: 512 * (ct + 1)],
                                lhsT=wk_sb[:, cc, 32 * h: 32 * (h + 1)],
                                rhs=xv,
                                start=(cc == 0), stop=(cc == 1),
                            )
                    for ct in range(4):
                        nc.vector.tensor_copy(
                            out=karr[h][32 * ct: 32 * (ct + 1), 512 * p_: 512 * (p_ + 1)],
                            in_=slab[0:32, 512 * ct: 512 * (ct + 1)],
                        )

            def v_build():
                # both heads at once: psum [128, 64*(k%..)] chunks
                slab = psA.tile([P, 2048], FP, tag="A")
                for k in range(NCH):
                    for cc in range(2):
                        nc.tensor.matmul(
                            slab[:, 64 * k: 64 * (k + 1)],
                            lhsT=xT[cc][:, P * k: P * (k + 1)],
                            rhs=wv_sb[:, cc, :],
                            start=(cc == 0), stop=(cc == 1),
                        )
                sv = slab[:].rearrange("p (k d) -> p k d", d=64)
                ones_st = bigp.tile([P, NCH], FP, tag="ones_st")
                nc.gpsimd.memset(ones_st[:], 1.0)
                for h in range(2):
                    vv = vaug[h][:].rearrange("p (k e) -> p k e", e=33)
                    nc.vector.tensor_copy(out=vv[:, :, 32], in_=ones_st[:])
                    nc.vector.tensor_copy(
                        out=vv[:, :, 0:32], in_=sv[:, :, 32 * h: 32 * (h + 1)]
                    )

            qrep_rounds(0)
            karr_build(0)
            v_build()

            # head-0 projection accumulator (filled during head-1 attention)
            yacc = bigp.tile([P, NCH * C], FP, tag="yacc")
            yv = yacc[:].rearrange("p (k c) -> p k c", c=C)

            def attention(h, post_it=None):
                vv = vaug[h][:].rearrange("p (k e) -> p k e", e=33)
                for it in range(ITILES):
                    i0 = 512 * it
                    av = psV.tile([P, 512], FP, tag="V")
                    cstart = 0
                    for gsz in GROUPS:
                        pool, tag, L = (psA, "A", 2048) if gsz == 4 else (psB, "B", 1536)
                        slab = pool.tile([P, L], FP, tag=tag)
                        for r in range(gsz):
                            c = cstart + r
                            rt = c % 4
                            nc.tensor.matmul(
                                slab[:, 512 * r: 512 * (r + 1)],
                                lhsT=karr[h][32 * rt: 32 * (rt + 1), P * (c // 4): P * (c // 4 + 1)],
                                rhs=qrep[h][32 * rt: 32 * (rt + 1), i0: i0 + 512],
                                start=True, stop=True,
                                tile_position=(32 * rt, 0),
                            )
                        eslab = expp.tile([P, L], FR, tag="E")
                        nc.scalar.activation(eslab[:], slab[:], AF.Exp)
                        for r in range(gsz):
                            c = cstart + r
                            nc.tensor.matmul(
                                av[0:33, :],
                                lhsT=vv[:, c, :],
                                rhs=eslab[:, 512 * r: 512 * (r + 1)],
                                start=(c == 0), stop=(c == NCH - 1),
                                skip_group_check=True,
                            )
                        cstart += gsz
                    nc.vector.tensor_copy(out=outT[32 * h: 32 * h + 32, i0: i0 + 512], in_=av[0:32, :])
                    nc.vector.tensor_copy(out=denrow[32 * h: 32 * h + 1, i0: i0 + 512], in_=av[32:33, :])
                    if post_it is not None:
                        post_it(it)

            def den_recip(h):
                # denominator row -> column layout, reciprocal
                dslab = psV.tile([P, 512], FP, tag="V")
                for t in range(NCH):
                    nc.tensor.transpose(
                        dslab[:, t: t + 1],
                        denrow[32 * h: 32 * h + 1, P * t: P * (t + 1)],
                        ident[32 * h: 32 * h + 1, 32 * h: 32 * h + 1],
                    )
                nc.vector.reciprocal(out=rden[:, 32 * h: 32 * h + 32], in_=dslab[:, 0:32])

            def y0_chunks(it):
                # head-0 output projection, interleaved into head-1 attention
                for k in range(4 * it, 4 * it + 4):
                    yp = psV.tile([P, 512], FP, tag="V")
                    nc.tensor.matmul(
                        yp[:, 0:C], lhsT=outT[0:32, P * k: P * (k + 1)],
                        rhs=wo_sb[0:32, :],
                        start=True, stop=True, tile_position=(0, 0),
                    )
                    nc.vector.tensor_scalar_mul(yv[:, k, :], yp[:, 0:C], rden[:, k: k + 1])

            attention(0)
            den_recip(0)
            qrep_rounds(1)
            karr_build(1)
            attention(1, post_it=y0_chunks)
            den_recip(1)

            # ---- tail: head-1 projection + combine + store ----
            for k in range(NCH):
                pool, tag, L = (psA, "A", 2048) if k % 2 == 0 else (psB, "B", 1536)
                yb = pool.tile([P, L], FP, tag=tag)
                nc.tensor.matmul(
                    yb[:, 0:C], lhsT=outT[32:64, P * k: P * (k + 1)],
                    rhs=wo_sb[32:64, :],
                    start=True, stop=True, tile_position=(32, 0),
                )
                yo = ytmpp.tile([P, C], FP, tag="yo")
                nc.vector.scalar_tensor_tensor(
                    out=yo[:], in0=yb[:, 0:C], scalar=rden[:, 32 + k: 33 + k],
                    in1=yv[:, k, :], op0=ALU.mult, op1=ALU.add,
                )
                dmae = nc.sync if k % 2 == 0 else nc.scalar
                dmae.dma_start(out=y_d[P * k: P * (k + 1), :], in_=yo[:])

    _split_excess_waits(nc, mybir)
    return nc


def _split_excess_waits(nc, mybir, maxw=1, carrier_cap=1):
    """walrus codegen allows few semaphore waits per engine instruction.

    Tile's scheduler can emit 3-4 on one matmul. Hoist the excess onto
    InstEventSemaphore carriers inserted immediately before the instruction
    on the same engine queue (queue is FIFO, so waiting in the carrier is
    equivalent; no reordering so no deadlock risk).
    """
    skip = {
        "InstEventSemaphore", "InstCall",
        "InstUnconditionalBranch", "InstISA", "InstRegisterMove",
    }
    for f in nc.m.functions:
        for blk in f.blocks:
            idx = 0
            while idx < len(blk.instructions):
                ins = blk.instructions[idx]
                si = getattr(ins, "sync_info", None)
                if (
                    si is not None and si.on_wait and len(si.on_wait) > maxw
                    and type(ins).__name__ not in skip
                ):
                    waits = list(si.on_wait)
                    keep, excess = waits[:maxw], waits[maxw:]
                    n_ins = 0
                    for i in range(0, len(excess), carrier_cap):
                        ev = mybir.InstEventSemaphore(
                            name=nc.get_next_instruction_name(),
                            engine=ins.engine,
                            ins=[], outs=[],
                            sync_info=mybir.SyncInfo(
                                on_wait=excess[i:i + carrier_cap], on_update=[]
                            ),
                        )
                        nc.register_instruction(ev)
                        blk.instructions.insert(idx + n_ins, ev)
                        n_ins += 1
                    ins.sync_info = mybir.SyncInfo(
                        on_wait=keep, on_update=list(si.on_update or [])
                    )
                    idx += n_ins
                idx += 1
    return nc


def get_nc():
    if "nc" not in _CACHED:
        _CACHED["nc"] = _build_nc()
    return _CACHED["nc"]


def make_in_maps(x, w_qkv, w_out):
    """Host-side sharding: core c -> batch c//2, heads (c%2)*2, (c%2)*2+1."""
    B = x.shape[0]
    xf = np.ascontiguousarray(x.reshape(B, N, C))
    scale = DH ** -0.5
    in_maps = []
    for core in range(8):
        b, hp = core // 2, core % 2
        h0, h1 = 2 * hp, 2 * hp + 1
        wq = np.concatenate(
            [np.tile(w_qkv[:, h * DH:(h + 1) * DH] * scale, (1, 4)) for h in (h0, h1)],
            axis=1,
        )  # [256, 256]
        wk = np.concatenate(
            [w_qkv[:, 128 + h * DH: 128 + (h + 1) * DH] for h in (h0, h1)], axis=1
        )  # [256, 64]
        wv = np.concatenate(
            [w_qkv[:, 256 + h * DH: 256 + (h + 1) * DH] for h in (h0, h1)], axis=1
        )  # [256, 64]
        wo = np.concatenate(
            [w_out[h * DH:(h + 1) * DH, :] for h in (h0, h1)], axis=0
        )  # [64, 256]
        in_maps.append({
            "x": np.ascontiguousarray(xf[b]),
            "wq": np.ascontiguousarray(wq.astype(np.float32)),
            "wk": np.ascontiguousarray(wk.astype(np.float32)),
            "wv": np.ascontiguousarray(wv.astype(np.float32)),
            "wo": np.ascontiguousarray(wo.astype(np.float32)),
        })
    return in_maps


def kernel(x, w_qkv, w_out, b_out):
    from concourse.bass_utils import run_bass_kernel_spmd

    nc = get_nc()
    in_maps = make_in_maps(
        np.asarray(x, dtype=np.float32),
        np.asarray(w_qkv, dtype=np.float32),
        np.asarray(w_out, dtype=np.float32),
    )
    res = run_bass_kernel_spmd(nc, in_maps, list(range(8))).results
    B, H, W = 4, 64, 64
    y = np.empty((B, N, C), dtype=np.float32)
    for b in range(B):
        y[b] = res[2 * b]["y"] + res[2 * b + 1]["y"]
    y += np.asarray(b_out, dtype=np.float32)
    return y.reshape(B, H, W, C)
